# revision 14
# baseline (speedup 1.0000x reference)
"""AttentionConv2D (3x3 windowed multi-head attention) on 8 TRN2 NeuronCores.

Sharding: data-parallel over batch (B=8 -> 1 image per core), weights replicated.
Per-core layout: channel-major [128 ch, 4096 pix]. All cross-channel/window
reductions and broadcasts run on the TensorEngine via block-structured matmuls;
per-pixel products run on DVE/ACT.

Math (host-folded):
  xn = (x - mu)/sqrt(var+eps) * g + b        (LayerNorm over channels)
  z  = (x - mu) * rstd;  xn = z*g + b  ->  fold g into W rows, b into biases.
  Q = z @ Wq' + bq'   (Wq' = diag(g) Wq, bq' = b@Wq + bq); same K', V'.
  scores[p,(n,k)] = (Q[p,n,:] . K[p+dk,n,:] + Q[p,n,:] . pos[k,n,:]) * A^-0.5
  pos-term folds to z @ Wqs + bqs (host-computed).
  attn = softmax_k(scores); out = sum_k attn_k * V_shift_k; final = out @ Wf + bf.

Device pipeline per core (z never materialized; aug rank-1 matmuls add the
-mu*rstd correction and biases):
  xs = x * bcast(rstd)          (bf16)
  Qp = Wq'.T @ xs (+aug)        etc.  -> PSUM, evict to SBUF bf16 (K,V padded)
  P_k = Q . K_pad(shift k)      (DVE bf16)
  scores_psum += BD_k.T @ P_k   (PE, accumulates on pos-scores)
  exp_s = exp(scores)           (ACT) -> SBUF
  denom36 = RS36.T @ exp_s      (PE: sum over k, replicated over k rows)
  attn = exp_s * recip(denom36) (DVE)
  rep_k = E_k.T @ attn          (PE: replicate head scores over 32 o-channels)
  m_k = rep_k * V_pad(shift k)  (DVE/ACT)
  final_psum += Wf.T @ m_k      (PE accumulates the sum over k)
  out = final_psum + bf -> DMA out.
"""

import math
import os
import sys

import numpy as np

sys.path.insert(0, "/opt/trn_rl_repo")

import ml_dtypes  # noqa: E402

BF16 = ml_dtypes.bfloat16

B, CIN, COUT, H, W, KS, NH = 8, 128, 128, 64, 64, 3, 4
A = CIN // NH          # 32
OSH = COUT // NH       # 32
K2 = KS * KS           # 9
NPIX = H * W           # 4096
PW = W + 2             # 66 padded width
PH = H + 2
NPAD = PW * PH + PW + 2  # 4356 + slack so shifted strided views stay in-bounds
NCHUNK = 8
CHUNK = NPIX // NCHUNK  # 512
ROWS_PER_CHUNK = H // NCHUNK  # 8
SCALE = A ** (-0.5)

_CACHE = {}


def _pos_encoding_np():
    pos = np.arange(K2, dtype=np.float32)[:, None]
    div = np.exp(np.arange(0, CIN, 2, dtype=np.float32) * (-math.log(10000.0) / CIN))
    ang = pos * div[None, :]
    return np.stack([np.sin(ang), np.cos(ang)], -1).reshape(K2, CIN)


def _host_fold(ln_g, ln_b, Wq, bq, Wk, bk, Wv, bv, Wp, bp, Wf, bf):
    """All weight-space precomputation (f64 for accuracy, cast at the end)."""
    g = ln_g.astype(np.float64)
    b = ln_b.astype(np.float64)
    Wq = Wq.astype(np.float64); Wk = Wk.astype(np.float64)
    Wv = Wv.astype(np.float64); Wp = Wp.astype(np.float64)
    Wf = Wf.astype(np.float64)
    bq = bq.astype(np.float64); bk = bk.astype(np.float64)
    bv = bv.astype(np.float64); bp = bp.astype(np.float64)
    bfv = bf.astype(np.float64)

    Wq_ = g[:, None] * Wq; bq_ = b @ Wq + bq
    Wk_ = g[:, None] * Wk; bk_ = b @ Wk + bk
    Wv_ = g[:, None] * Wv; bv_ = b @ Wv + bv

    pos = _pos_encoding_np().astype(np.float64) @ Wp + bp  # [K2, NH*A]
    pos = pos.reshape(K2, NH, A)

    # pos-scores: row layout (n,k) = n*9+k ; scores_pos = z @ Wqs + bqs, scaled
    Wqs = np.zeros((CIN, NH * K2))
    bqs = np.zeros((NH * K2,))
    Wq_r = Wq_.reshape(CIN, NH, A)
    bq_r = bq_.reshape(NH, A)
    for n in range(NH):
        for k in range(K2):
            Wqs[:, n * K2 + k] = Wq_r[:, n, :] @ pos[k, n, :]
            bqs[n * K2 + k] = bq_r[n, :] @ pos[k, n, :]
    Wqs *= SCALE
    bqs *= SCALE

    # BD_k [CIN, 36]: (n,a) x (n*9+k) = SCALE ; concat over k -> [128, 9*36]
    bd = np.zeros((K2, CIN, NH * K2))
    for k in range(K2):
        for n in range(NH):
            bd[k, n * A:(n + 1) * A, n * K2 + k] = SCALE
    bd = np.concatenate([bd[k] for k in range(K2)], axis=1)  # [128, 324]

    # E_k [36, 128]: (n*9+k') x (n,o) = 1 iff k'==k ; concat -> [36, 9*128]
    ek = np.zeros((K2, NH * K2, CIN))
    for k in range(K2):
        for n in range(NH):
            ek[k, n * K2 + k, n * OSH:(n + 1) * OSH] = 1.0
    ek = np.concatenate([ek[k] for k in range(K2)], axis=1)  # [36, 1152]

    # RS36 [36, 36]: (n*9+k) x (n'*9+k') = 1 iff n==n'  (sum over k, rep over k')
    rs = np.zeros((NH * K2, NH * K2))
    for n in range(NH):
        rs[n * K2:(n + 1) * K2, n * K2:(n + 1) * K2] = 1.0

    # aug lhsT rows: [colsum(W'); bias] per projection, concat cols: q,k,v,qs
    def aug(Wm, bm):
        return np.stack([Wm.sum(axis=0), bm], axis=0)  # [2, M]

    waug = np.concatenate(
        [aug(Wq_, bq_), aug(Wk_, bk_), aug(Wv_, bv_), aug(Wqs, bqs)], axis=1
    )  # [2, 128*3+36]

    c = {
        "wq": Wq_.astype(BF16), "wk": Wk_.astype(BF16), "wv": Wv_.astype(BF16),
        "wqs": Wqs.astype(BF16), "waug": waug.astype(BF16),
        "bd": bd.astype(BF16), "ek": ek.astype(BF16), "rs": rs.astype(BF16),
        "wf": Wf.astype(BF16),
        "bfb": bfv.astype(np.float32).reshape(COUT, 1),
        "ones_k": np.ones((CIN, 1), dtype=BF16),
        "ones_m": np.ones((1, CIN), dtype=BF16),
        "ones_row": np.ones((1, NPIX), dtype=BF16),
    }
    return c


def _shift_delta(k):
    di, dj = k // KS - 1, k % KS - 1
    return di * PW + dj


def _build_bass():
    import concourse.bass as bass
    import concourse.tile as tile
    from concourse import bacc, mybir

    f32 = mybir.dt.float32
    bf16 = mybir.dt.bfloat16
    AF = mybir.ActivationFunctionType

    nc = bacc.Bacc("TRN2", target_bir_lowering=False, debug=False)

    x_ext = nc.dram_tensor("x", [CIN, NPIX], f32, kind="ExternalInput")
    wq_ext = nc.dram_tensor("wq", [CIN, CIN], bf16, kind="ExternalInput")
    wk_ext = nc.dram_tensor("wk", [CIN, CIN], bf16, kind="ExternalInput")
    wv_ext = nc.dram_tensor("wv", [CIN, CIN], bf16, kind="ExternalInput")
    wqs_ext = nc.dram_tensor("wqs", [CIN, NH * K2], bf16, kind="ExternalInput")
    waug_ext = nc.dram_tensor("waug", [2, 3 * CIN + NH * K2], bf16, kind="ExternalInput")
    bd_ext = nc.dram_tensor("bd", [CIN, K2 * NH * K2], bf16, kind="ExternalInput")
    ek_ext = nc.dram_tensor("ek", [NH * K2, K2 * CIN], bf16, kind="ExternalInput")
    rs_ext = nc.dram_tensor("rs", [NH * K2, NH * K2], bf16, kind="ExternalInput")
    wf_ext = nc.dram_tensor("wf", [COUT, COUT], bf16, kind="ExternalInput")
    bfb_ext = nc.dram_tensor("bfb", [COUT, 1], f32, kind="ExternalInput")
    onek_ext = nc.dram_tensor("ones_k", [CIN, 1], bf16, kind="ExternalInput")
    onem_ext = nc.dram_tensor("ones_m", [1, CIN], bf16, kind="ExternalInput")
    oner_ext = nc.dram_tensor("ones_row", [1, NPIX], bf16, kind="ExternalInput")
    out_ext = nc.dram_tensor("out", [COUT, NPIX], f32, kind="ExternalOutput")

    with tile.TileContext(nc) as tc:
        _kernel_body(tc, nc, mybir, f32, bf16, AF, bass,
                     x_ext, wq_ext, wk_ext, wv_ext, wqs_ext, waug_ext, bd_ext,
                     ek_ext, rs_ext, wf_ext, bfb_ext, onek_ext, onem_ext,
                     oner_ext, out_ext)

    nc.compile()
    return nc


def _kernel_body(tc, nc, mybir, f32, bf16, AF, bass,
                 x_ext, wq_ext, wk_ext, wv_ext, wqs_ext, waug_ext, bd_ext,
                 ek_ext, rs_ext, wf_ext, bfb_ext, onek_ext, onem_ext,
                 oner_ext, out_ext):
    from contextlib import ExitStack

    ctx = ExitStack()
    with ctx:
        consts = ctx.enter_context(tc.tile_pool(name="consts", bufs=1))
        big = ctx.enter_context(tc.tile_pool(name="big", bufs=1))
        mkp = ctx.enter_context(tc.tile_pool(name="mk", bufs=4))
        mallp = ctx.enter_context(tc.tile_pool(name="mall", bufs=2))
        chkp = ctx.enter_context(tc.tile_pool(name="chk", bufs=8))
        smallp = ctx.enter_context(tc.tile_pool(name="small", bufs=1))
        dramp = ctx.enter_context(tc.tile_pool(name="drams", bufs=1, space="DRAM"))
        ps128 = ctx.enter_context(tc.tile_pool(name="ps128", bufs=3, space="PSUM"))
        psacc = ctx.enter_context(tc.tile_pool(name="psacc", bufs=2, space="PSUM"))
        ps36 = ctx.enter_context(tc.tile_pool(name="ps36", bufs=3, space="PSUM"))

        def mm(out, lhsT, rhs, **kw):
            nc.tensor.matmul(out, lhsT, rhs, **kw)

        # ---- load constants ----
        wq = consts.tile([CIN, CIN], bf16); nc.sync.dma_start(wq[:], wq_ext[:])
        wk = consts.tile([CIN, CIN], bf16); nc.sync.dma_start(wk[:], wk_ext[:])
        wv = consts.tile([CIN, CIN], bf16); nc.sync.dma_start(wv[:], wv_ext[:])
        wqs = consts.tile([CIN, NH * K2], bf16); nc.sync.dma_start(wqs[:], wqs_ext[:])
        waug = consts.tile([2, 3 * CIN + NH * K2], bf16)
        nc.sync.dma_start(waug[:], waug_ext[:])
        bdw = consts.tile([CIN, K2 * NH * K2], bf16); nc.sync.dma_start(bdw[:], bd_ext[:])
        ekw = consts.tile([NH * K2, K2 * CIN], bf16); nc.sync.dma_start(ekw[:], ek_ext[:])
        rsw = consts.tile([NH * K2, NH * K2], bf16); nc.sync.dma_start(rsw[:], rs_ext[:])
        wf = consts.tile([COUT, COUT], bf16); nc.sync.dma_start(wf[:], wf_ext[:])
        bfb = consts.tile([COUT, 1], f32); nc.sync.dma_start(bfb[:], bfb_ext[:])
        ones_k = consts.tile([CIN, 1], bf16); nc.sync.dma_start(ones_k[:], onek_ext[:])
        ones_m = consts.tile([1, CIN], bf16); nc.sync.dma_start(ones_m[:], onem_ext[:])

        # ---- input image ----
        x_sb = big.tile([CIN, NPIX], f32)
        nc.sync.dma_start(x_sb[:], x_ext[:])

        # ---- stats: S1 = sum_c x, S2 = sum_c x^2 (bf16 inputs, fp32 psum) ----
        s1_rows = smallp.tile([1, NPIX], f32, tag="s1_rows")
        s2_rows = smallp.tile([1, NPIX], f32, tag="s2_rows")
        for c in range(NCHUNK):
            sl = slice(c * CHUNK, (c + 1) * CHUNK)
            x_bf = mkp.tile([CIN, CHUNK], bf16, tag="xbf")
            nc.gpsimd.tensor_copy(x_bf[:], x_sb[:, sl])
            sq_bf = mkp.tile([CIN, CHUNK], bf16, tag="sqbf")
            nc.gpsimd.tensor_tensor(sq_bf[:], x_sb[:, sl], x_sb[:, sl],
                                    mybir.AluOpType.mult)
            s1 = ps36.tile([1, CHUNK], f32, tag="ps36")
            mm(s1[:], ones_k[:], x_bf[:], start=True, stop=True)
            s2 = ps36.tile([1, CHUNK], f32, tag="ps36")
            mm(s2[:], ones_k[:], sq_bf[:], start=True, stop=True)
            nc.vector.tensor_copy(s1_rows[0:1, sl], s1[:])
            nc.scalar.copy(s2_rows[0:1, sl], s2[:])

        # ---- pack stats via DRAM bounce -> [128, 64] ----
        s_dram = dramp.tile([2, NPIX], f32)
        nc.sync.dma_start(s_dram[0:1, :], s1_rows[:])
        nc.sync.dma_start(s_dram[1:2, :], s2_rows[:])
        s_pack = smallp.tile([CIN, 2 * NPIX // CIN], f32, tag="s_pack")  # [128, 64]
        PCK = NPIX // CIN  # 32
        nc.sync.dma_start(s_pack[:, 0:PCK], s_dram[0, :].rearrange("(p j) -> p j", p=CIN))
        nc.sync.dma_start(s_pack[:, PCK:2 * PCK], s_dram[1, :].rearrange("(p j) -> p j", p=CIN))

        # rstd = 1/sqrt(S2/128 - (S1/128)^2 + eps); sneg = -(S1/128)*rstd
        S1 = s_pack[:, 0:PCK]
        S2 = s_pack[:, PCK:2 * PCK]
        stat2 = smallp.tile([CIN, 4 * PCK], f32, tag="stat2")
        mean = stat2[:, 0:PCK]
        var = stat2[:, PCK:2 * PCK]
        rstd = stat2[:, 2 * PCK:3 * PCK]
        sneg = stat2[:, 3 * PCK:4 * PCK]
        nc.vector.tensor_scalar_mul(mean[:], S1[:], 1.0 / CIN)
        nc.vector.tensor_tensor(var[:], mean[:], mean[:], mybir.AluOpType.mult)
        nc.vector.tensor_scalar_mul(S2[:], S2[:], 1.0 / CIN)
        nc.vector.tensor_tensor(var[:], S2[:], var[:], mybir.AluOpType.subtract)
        nc.vector.tensor_scalar_add(var[:], var[:], 1e-5)
        nc.scalar.sqrt(var[:], var[:])              # std
        nc.vector.reciprocal_approx_fast(rstd[:], var[:])
        nc.vector.tensor_tensor(sneg[:], mean[:], rstd[:], mybir.AluOpType.mult)
        nc.vector.tensor_scalar_mul(sneg[:], sneg[:], -1.0)
        stat_bf = smallp.tile([CIN, 2 * PCK], bf16, tag="stat_bf")
        nc.vector.tensor_copy(stat_bf[:, 0:PCK], rstd[:])
        nc.vector.tensor_copy(stat_bf[:, PCK:2 * PCK], sneg[:])

        # unpack to rows via DRAM bounce
        r_dram = dramp.tile([2 * PCK * CIN], bf16)
        nc.sync.dma_start(r_dram[0:PCK * CIN].rearrange("(p j) -> p j", p=CIN),
                          stat_bf[:, 0:PCK])
        nc.sync.dma_start(r_dram[PCK * CIN:].rearrange("(p j) -> p j", p=CIN),
                          stat_bf[:, PCK:2 * PCK])
        rstd_row = smallp.tile([1, NPIX], bf16, tag="rstd_row")
        nc.sync.dma_start(rstd_row[:], r_dram[0:NPIX].rearrange("(o p) -> o p", o=1))
        srow2 = smallp.tile([2, NPIX], bf16, tag="srow2")
        nc.sync.dma_start(srow2[0:1, :], r_dram[NPIX:].rearrange("(o p) -> o p", o=1))
        nc.sync.dma_start(srow2[1:2, :], oner_ext[:])

        # ---- padded K/V buffers (zero borders) ----
        k_pad = big.tile([CIN, NPAD], bf16)
        v_pad = big.tile([CIN, NPAD], bf16)
        nc.gpsimd.memset(k_pad[:], 0.0)
        nc.gpsimd.memset(v_pad[:], 0.0)
        q_tiles = [None] * NCHUNK
        xs_tiles = [None] * NCHUNK

        def pad_view(t, c, delta=0):
            # rows c*8 .. c*8+7 of padded buffer, inner 64 cols, shifted by delta
            off = (1 + c * ROWS_PER_CHUNK) * PW + 1 + delta
            return t[:, off:off + ROWS_PER_CHUNK * PW].rearrange(
                "p (r w) -> p r w", r=ROWS_PER_CHUNK, w=PW)[:, :, 0:W]

        AUGQ, AUGK, AUGV, AUGS = (slice(0, CIN), slice(CIN, 2 * CIN),
                                  slice(2 * CIN, 3 * CIN),
                                  slice(3 * CIN, 3 * CIN + NH * K2))

        # ---- phase 2: normalize + projections (fills q_sb, k_pad, v_pad) ----
        for c in range(NCHUNK):
            sl = slice(c * CHUNK, (c + 1) * CHUNK)
            rb = ps128.tile([CIN, CHUNK], f32, tag="ps128")
            mm(rb[:], ones_m[:], rstd_row[:, sl], start=True, stop=True)
            xs_c = chkp.tile([CIN, CHUNK], bf16, tag="xs")
            xs_tiles[c] = xs_c
            nc.vector.tensor_tensor(xs_c[:], x_sb[:, sl], rb[:], mybir.AluOpType.mult)

            qp = ps128.tile([CIN, CHUNK], f32, tag="ps128")
            mm(qp[:], wq[:], xs_c[:], start=True, stop=False)
            mm(qp[:], waug[:, AUGQ], srow2[:, sl], start=False, stop=True)
            q_c = chkp.tile([CIN, CHUNK], bf16, tag="q")
            q_tiles[c] = q_c
            nc.vector.tensor_copy(q_c[:], qp[:])

            kp = ps128.tile([CIN, CHUNK], f32, tag="ps128")
            mm(kp[:], wk[:], xs_c[:], start=True, stop=False)
            mm(kp[:], waug[:, AUGK], srow2[:, sl], start=False, stop=True)
            nc.vector.tensor_copy(pad_view(k_pad, c)[:], kp[:].rearrange(
                "p (r w) -> p r w", r=ROWS_PER_CHUNK, w=W))

            vp = ps128.tile([CIN, CHUNK], f32, tag="ps128")
            mm(vp[:], wv[:], xs_c[:], start=True, stop=False)
            mm(vp[:], waug[:, AUGV], srow2[:, sl], start=False, stop=True)
            nc.scalar.copy(pad_view(v_pad, c)[:], vp[:].rearrange(
                "p (r w) -> p r w", r=ROWS_PER_CHUNK, w=W))

        # ---- phase 3+4 per chunk: scores, softmax, AV, Wf, out ----
        for c in range(NCHUNK):
            sl = slice(c * CHUNK, (c + 1) * CHUNK)
            q_v = q_tiles[c][:].rearrange("p (r w) -> p r w", r=ROWS_PER_CHUNK, w=W)

            sc = ps36.tile([NH * K2, CHUNK], f32, tag="ps36")
            mm(sc[:], wqs[:], xs_tiles[c][:], start=True, stop=False)
            mm(sc[:], waug[:, AUGS], srow2[:, sl], start=False, stop=False)
            for k in range(K2):
                pk = mkp.tile([CIN, CHUNK], bf16, tag="pk")
                pk_v = pk[:].rearrange("p (r w) -> p r w", r=ROWS_PER_CHUNK, w=W)
                eng = nc.gpsimd if k in (0, 5) else nc.vector
                eng.tensor_tensor(pk_v[:], q_v[:],
                                  pad_view(k_pad, c, _shift_delta(k))[:],
                                  mybir.AluOpType.mult)
                mm(sc[:], bdw[:, k * NH * K2:(k + 1) * NH * K2], pk[:],
                   start=False, stop=(k == K2 - 1))

            exp_c = chkp.tile([NH * K2, CHUNK], bf16, tag="exp")
            nc.scalar.activation(exp_c[:], sc[:], AF.Exp)
            dn = ps36.tile([NH * K2, CHUNK], f32, tag="ps36")
            mm(dn[:], rsw[:], exp_c[:], start=True, stop=True)
            rcp36 = mkp.tile([NH * K2, CHUNK], f32, tag="rcp")
            nc.vector.reciprocal_approx_fast(rcp36[:], dn[:])
            attn_c = chkp.tile([NH * K2, CHUNK], bf16, tag="attn")
            nc.vector.tensor_tensor(attn_c[:], exp_c[:], rcp36[:],
                                    mybir.AluOpType.mult)

            acc = psacc.tile([COUT, CHUNK], f32, tag="acc")
            m_all = mallp.tile([CIN, K2 * CHUNK], bf16, tag="mall")
            for k in range(K2):
                rep = ps128.tile([CIN, CHUNK], f32, tag="ps128")
                mm(rep[:], ekw[:, k * CIN:(k + 1) * CIN], attn_c[:],
                   start=True, stop=True)
                rep_sb = mkp.tile([CIN, CHUNK], bf16, tag="repsb")
                nc.scalar.copy(rep_sb[:], rep[:])
                mk_v = m_all[:, k * CHUNK:(k + 1) * CHUNK].rearrange(
                    "p (r w) -> p r w", r=ROWS_PER_CHUNK, w=W)
                nc.vector.tensor_tensor(
                    mk_v[:], rep_sb[:].rearrange("p (r w) -> p r w",
                                                 r=ROWS_PER_CHUNK, w=W),
                    pad_view(v_pad, c, _shift_delta(k))[:],
                    mybir.AluOpType.mult)
                mm(acc[:], wf[:], m_all[:, k * CHUNK:(k + 1) * CHUNK],
                   start=(k == 0), stop=(k == K2 - 1))
            out_sb = mkp.tile([COUT, CHUNK], f32, tag="outsb")
            nc.vector.tensor_scalar_add(out_sb[:], acc[:], bfb[:])
            nc.sync.dma_start(out_ext[:, sl], out_sb[:])


def _get_compiled():
    if "nc" not in _CACHE:
        _CACHE["nc"] = _build_bass()
    return _CACHE["nc"]


def kernel(**inputs):
    x = np.asarray(inputs["x"], dtype=np.float32)          # [B, CIN, H, W]
    consts = _host_fold(
        np.asarray(inputs["ln_g"]), np.asarray(inputs["ln_b"]),
        np.asarray(inputs["Wq"]), np.asarray(inputs["bq"]),
        np.asarray(inputs["Wk"]), np.asarray(inputs["bk"]),
        np.asarray(inputs["Wv"]), np.asarray(inputs["bv"]),
        np.asarray(inputs["Wp"]), np.asarray(inputs["bp"]),
        np.asarray(inputs["Wf"]), np.asarray(inputs["bf"]),
    )

    nc = _get_compiled()

    from concourse.bass_utils import run_bass_kernel_spmd

    core_ids = list(range(B))
    in_maps = []
    for i in range(B):
        m = {"x": np.ascontiguousarray(x[i].reshape(CIN, NPIX))}
        m.update(consts)
        in_maps.append(m)

    res = run_bass_kernel_spmd(nc, in_maps, core_ids,
                               trace=bool(int(os.environ.get("KTRACE", "0"))))
    _CACHE["last_result"] = res
    out = np.stack([res.results[i]["out"].reshape(COUT, H, W) for i in range(B)])
    return out.astype(np.float32)


if __name__ == "__main__":
    # smoke build only
    nc = _get_compiled()
    print("compiled OK")


# revision 17
# speedup vs baseline: 1.1145x; 1.1145x over previous
"""AttentionConv2D (3x3 windowed multi-head attention) on 8 TRN2 NeuronCores.

Sharding: data-parallel over batch (B=8 -> 1 image per core), weights replicated.
Per-core layout: channel-major [128 ch, 4096 pix]. All cross-channel/window
reductions and broadcasts run on the TensorEngine via block-structured matmuls;
per-pixel products run on DVE/ACT.

Math (host-folded):
  xn = (x - mu)/sqrt(var+eps) * g + b        (LayerNorm over channels)
  z  = (x - mu) * rstd;  xn = z*g + b  ->  fold g into W rows, b into biases.
  Q = z @ Wq' + bq'   (Wq' = diag(g) Wq, bq' = b@Wq + bq); same K', V'.
  scores[p,(n,k)] = (Q[p,n,:] . K[p+dk,n,:] + Q[p,n,:] . pos[k,n,:]) * A^-0.5
  pos-term folds to z @ Wqs + bqs (host-computed).
  attn = softmax_k(scores); out = sum_k attn_k * V_shift_k; final = out @ Wf + bf.

Device pipeline per core (z never materialized; aug rank-1 matmuls add the
-mu*rstd correction and biases):
  xs = x * bcast(rstd)          (bf16)
  Qp = Wq'.T @ xs (+aug)        etc.  -> PSUM, evict to SBUF bf16 (K,V padded)
  P_k = Q . K_pad(shift k)      (DVE bf16)
  scores_psum += BD_k.T @ P_k   (PE, accumulates on pos-scores)
  exp_s = exp(scores)           (ACT) -> SBUF
  denom36 = RS36.T @ exp_s      (PE: sum over k, replicated over k rows)
  attn = exp_s * recip(denom36) (DVE)
  rep_k = E_k.T @ attn          (PE: replicate head scores over 32 o-channels)
  m_k = rep_k * V_pad(shift k)  (DVE/ACT)
  final_psum += Wf.T @ m_k      (PE accumulates the sum over k)
  out = final_psum + bf -> DMA out.
"""

import math
import os
import sys

import numpy as np

sys.path.insert(0, "/opt/trn_rl_repo")

import ml_dtypes  # noqa: E402

BF16 = ml_dtypes.bfloat16

B, CIN, COUT, H, W, KS, NH = 8, 128, 128, 64, 64, 3, 4
A = CIN // NH          # 32
OSH = COUT // NH       # 32
K2 = KS * KS           # 9
NPIX = H * W           # 4096
PW = W + 2             # 66 padded width
PH = H + 2
NPAD = PW * PH + PW + 2  # 4356 + slack so shifted strided views stay in-bounds
NCHUNK = 8
CHUNK = NPIX // NCHUNK  # 512
ROWS_PER_CHUNK = H // NCHUNK  # 8
SCALE = A ** (-0.5)

_CACHE = {}


def _pos_encoding_np():
    pos = np.arange(K2, dtype=np.float32)[:, None]
    div = np.exp(np.arange(0, CIN, 2, dtype=np.float32) * (-math.log(10000.0) / CIN))
    ang = pos * div[None, :]
    return np.stack([np.sin(ang), np.cos(ang)], -1).reshape(K2, CIN)


def _host_fold(ln_g, ln_b, Wq, bq, Wk, bk, Wv, bv, Wp, bp, Wf, bf):
    """All weight-space precomputation (f64 for accuracy, cast at the end)."""
    g = ln_g.astype(np.float64)
    b = ln_b.astype(np.float64)
    Wq = Wq.astype(np.float64); Wk = Wk.astype(np.float64)
    Wv = Wv.astype(np.float64); Wp = Wp.astype(np.float64)
    Wf = Wf.astype(np.float64)
    bq = bq.astype(np.float64); bk = bk.astype(np.float64)
    bv = bv.astype(np.float64); bp = bp.astype(np.float64)
    bfv = bf.astype(np.float64)

    Wq_ = g[:, None] * Wq; bq_ = b @ Wq + bq
    Wk_ = g[:, None] * Wk; bk_ = b @ Wk + bk
    Wv_ = g[:, None] * Wv; bv_ = b @ Wv + bv

    pos = _pos_encoding_np().astype(np.float64) @ Wp + bp  # [K2, NH*A]
    pos = pos.reshape(K2, NH, A)

    # pos-scores: row layout (n,k) = n*9+k ; scores_pos = z @ Wqs + bqs, scaled
    Wqs = np.zeros((CIN, NH * K2))
    bqs = np.zeros((NH * K2,))
    Wq_r = Wq_.reshape(CIN, NH, A)
    bq_r = bq_.reshape(NH, A)
    for n in range(NH):
        for k in range(K2):
            Wqs[:, n * K2 + k] = Wq_r[:, n, :] @ pos[k, n, :]
            bqs[n * K2 + k] = bq_r[n, :] @ pos[k, n, :]
    Wqs *= SCALE
    bqs *= SCALE

    # BD_k [CIN, 36]: (n,a) x (n*9+k) = SCALE ; concat over k -> [128, 9*36]
    bd = np.zeros((K2, CIN, NH * K2))
    for k in range(K2):
        for n in range(NH):
            bd[k, n * A:(n + 1) * A, n * K2 + k] = SCALE
    bd = np.concatenate([bd[k] for k in range(K2)], axis=1)  # [128, 324]

    # E_k [36, 128]: (n*9+k') x (n,o) = 1 iff k'==k ; concat -> [36, 9*128]
    ek = np.zeros((K2, NH * K2, CIN))
    for k in range(K2):
        for n in range(NH):
            ek[k, n * K2 + k, n * OSH:(n + 1) * OSH] = 1.0
    ek = np.concatenate([ek[k] for k in range(K2)], axis=1)  # [36, 1152]

    # RS36 [36, 36]: (n*9+k) x (n'*9+k') = 1 iff n==n'  (sum over k, rep over k')
    rs = np.zeros((NH * K2, NH * K2))
    for n in range(NH):
        rs[n * K2:(n + 1) * K2, n * K2:(n + 1) * K2] = 1.0

    # aug lhsT rows: [colsum(W'); bias] per projection, concat cols: q,k,v,qs
    def aug(Wm, bm):
        return np.stack([Wm.sum(axis=0), bm], axis=0)  # [2, M]

    waug = np.concatenate(
        [aug(Wq_, bq_), aug(Wk_, bk_), aug(Wv_, bv_), aug(Wqs, bqs)], axis=1
    )  # [2, 128*3+36]

    c = {
        "wq": Wq_.astype(BF16), "wk": Wk_.astype(BF16), "wv": Wv_.astype(BF16),
        "wqs": Wqs.astype(BF16), "waug": waug.astype(BF16),
        "bd": bd.astype(BF16), "ek": ek.astype(BF16), "rs": rs.astype(BF16),
        "wf": Wf.astype(BF16),
        "bfb": bfv.astype(np.float32).reshape(COUT, 1),
        "ones_k": np.ones((CIN, 1), dtype=BF16),
        "ones_m": np.ones((1, CIN), dtype=BF16),
        "ones_row": np.ones((1, NPIX), dtype=BF16),
    }
    return c


def _shift_delta(k):
    di, dj = k // KS - 1, k % KS - 1
    return di * PW + dj


def _build_bass():
    import concourse.bass as bass
    import concourse.tile as tile
    from concourse import bacc, mybir

    f32 = mybir.dt.float32
    bf16 = mybir.dt.bfloat16
    AF = mybir.ActivationFunctionType

    nc = bacc.Bacc("TRN2", target_bir_lowering=False, debug=False)

    x_ext = nc.dram_tensor("x", [CIN, NPIX], f32, kind="ExternalInput")
    wq_ext = nc.dram_tensor("wq", [CIN, CIN], bf16, kind="ExternalInput")
    wk_ext = nc.dram_tensor("wk", [CIN, CIN], bf16, kind="ExternalInput")
    wv_ext = nc.dram_tensor("wv", [CIN, CIN], bf16, kind="ExternalInput")
    wqs_ext = nc.dram_tensor("wqs", [CIN, NH * K2], bf16, kind="ExternalInput")
    waug_ext = nc.dram_tensor("waug", [2, 3 * CIN + NH * K2], bf16, kind="ExternalInput")
    bd_ext = nc.dram_tensor("bd", [CIN, K2 * NH * K2], bf16, kind="ExternalInput")
    ek_ext = nc.dram_tensor("ek", [NH * K2, K2 * CIN], bf16, kind="ExternalInput")
    rs_ext = nc.dram_tensor("rs", [NH * K2, NH * K2], bf16, kind="ExternalInput")
    wf_ext = nc.dram_tensor("wf", [COUT, COUT], bf16, kind="ExternalInput")
    bfb_ext = nc.dram_tensor("bfb", [COUT, 1], f32, kind="ExternalInput")
    onek_ext = nc.dram_tensor("ones_k", [CIN, 1], bf16, kind="ExternalInput")
    onem_ext = nc.dram_tensor("ones_m", [1, CIN], bf16, kind="ExternalInput")
    oner_ext = nc.dram_tensor("ones_row", [1, NPIX], bf16, kind="ExternalInput")
    out_ext = nc.dram_tensor("out", [COUT, NPIX], f32, kind="ExternalOutput")

    with tile.TileContext(nc) as tc:
        _kernel_body(tc, nc, mybir, f32, bf16, AF, bass,
                     x_ext, wq_ext, wk_ext, wv_ext, wqs_ext, waug_ext, bd_ext,
                     ek_ext, rs_ext, wf_ext, bfb_ext, onek_ext, onem_ext,
                     oner_ext, out_ext)

    nc.compile()
    return nc


def _kernel_body(tc, nc, mybir, f32, bf16, AF, bass,
                 x_ext, wq_ext, wk_ext, wv_ext, wqs_ext, waug_ext, bd_ext,
                 ek_ext, rs_ext, wf_ext, bfb_ext, onek_ext, onem_ext,
                 oner_ext, out_ext):
    from contextlib import ExitStack

    ctx = ExitStack()
    with ctx:
        consts = ctx.enter_context(tc.tile_pool(name="consts", bufs=1))
        big = ctx.enter_context(tc.tile_pool(name="big", bufs=1))
        mkp = ctx.enter_context(tc.tile_pool(name="mk", bufs=4))
        mallp = ctx.enter_context(tc.tile_pool(name="mall", bufs=2))
        chkp = ctx.enter_context(tc.tile_pool(name="chk", bufs=8))
        smallp = ctx.enter_context(tc.tile_pool(name="small", bufs=1))
        dramp = ctx.enter_context(tc.tile_pool(name="drams", bufs=1, space="DRAM"))
        ps128 = ctx.enter_context(tc.tile_pool(name="ps128", bufs=2, space="PSUM"))
        psacc = ctx.enter_context(tc.tile_pool(name="psacc", bufs=2, space="PSUM"))
        ps36 = ctx.enter_context(tc.tile_pool(name="ps36", bufs=4, space="PSUM"))

        def mm(out, lhsT, rhs, **kw):
            nc.tensor.matmul(out, lhsT, rhs, **kw)

        # ---- load constants ----
        wq = consts.tile([CIN, CIN], bf16); nc.sync.dma_start(wq[:], wq_ext[:])
        wk = consts.tile([CIN, CIN], bf16); nc.sync.dma_start(wk[:], wk_ext[:])
        wv = consts.tile([CIN, CIN], bf16); nc.sync.dma_start(wv[:], wv_ext[:])
        wqs = consts.tile([CIN, NH * K2], bf16); nc.sync.dma_start(wqs[:], wqs_ext[:])
        waug = consts.tile([2, 3 * CIN + NH * K2], bf16)
        nc.sync.dma_start(waug[:], waug_ext[:])
        bdw = consts.tile([CIN, K2 * NH * K2], bf16); nc.sync.dma_start(bdw[:], bd_ext[:])
        ekw = consts.tile([NH * K2, K2 * CIN], bf16); nc.sync.dma_start(ekw[:], ek_ext[:])
        rsw = consts.tile([NH * K2, NH * K2], bf16); nc.sync.dma_start(rsw[:], rs_ext[:])
        wf = consts.tile([COUT, COUT], bf16); nc.sync.dma_start(wf[:], wf_ext[:])
        bfb = consts.tile([COUT, 1], f32); nc.sync.dma_start(bfb[:], bfb_ext[:])
        ones_k = consts.tile([CIN, 1], bf16); nc.sync.dma_start(ones_k[:], onek_ext[:])
        ones_m = consts.tile([1, CIN], bf16); nc.sync.dma_start(ones_m[:], onem_ext[:])

        # ---- input image ----
        x_sb = big.tile([CIN, NPIX], f32)
        QT = NPIX // 4
        for _i in range(4):
            nc.sync.dma_start(x_sb[:, _i * QT:(_i + 1) * QT],
                              x_ext[:, _i * QT:(_i + 1) * QT])

        # ---- stats: S1 = sum_c x, S2 = sum_c x^2 (bf16 inputs, fp32 psum) ----
        s1_rows = smallp.tile([1, NPIX], f32, tag="s1_rows")
        s2_rows = smallp.tile([1, NPIX], f32, tag="s2_rows")
        for c in range(NCHUNK):
            sl = slice(c * CHUNK, (c + 1) * CHUNK)
            x_bf = mkp.tile([CIN, CHUNK], bf16, tag="xbf")
            nc.gpsimd.tensor_copy(x_bf[:], x_sb[:, sl])
            sq_bf = mkp.tile([CIN, CHUNK], bf16, tag="sqbf")
            nc.gpsimd.tensor_tensor(sq_bf[:], x_sb[:, sl], x_sb[:, sl],
                                    mybir.AluOpType.mult)
            s1 = ps36.tile([1, CHUNK], f32, tag="ps36")
            mm(s1[:], ones_k[:], x_bf[:], start=True, stop=True)
            s2 = ps36.tile([1, CHUNK], f32, tag="ps36")
            mm(s2[:], ones_k[:], sq_bf[:], start=True, stop=True)
            nc.vector.tensor_copy(s1_rows[0:1, sl], s1[:])
            nc.scalar.copy(s2_rows[0:1, sl], s2[:])

        # ---- pack stats via DRAM bounce -> [128, 64] ----
        s_dram = dramp.tile([2, NPIX], f32)
        nc.sync.dma_start(s_dram[0:1, :], s1_rows[:])
        nc.sync.dma_start(s_dram[1:2, :], s2_rows[:])
        s_pack = smallp.tile([CIN, 2 * NPIX // CIN], f32, tag="s_pack")  # [128, 64]
        PCK = NPIX // CIN  # 32
        nc.sync.dma_start(s_pack[:, 0:PCK], s_dram[0, :].rearrange("(p j) -> p j", p=CIN))
        nc.sync.dma_start(s_pack[:, PCK:2 * PCK], s_dram[1, :].rearrange("(p j) -> p j", p=CIN))

        # rstd = 1/sqrt(S2/128 - (S1/128)^2 + eps); sneg = -(S1/128)*rstd
        S1 = s_pack[:, 0:PCK]
        S2 = s_pack[:, PCK:2 * PCK]
        stat2 = smallp.tile([CIN, 4 * PCK], f32, tag="stat2")
        mean = stat2[:, 0:PCK]
        var = stat2[:, PCK:2 * PCK]
        rstd = stat2[:, 2 * PCK:3 * PCK]
        sneg = stat2[:, 3 * PCK:4 * PCK]
        nc.vector.tensor_scalar_mul(mean[:], S1[:], 1.0 / CIN)
        nc.vector.tensor_tensor(var[:], mean[:], mean[:], mybir.AluOpType.mult)
        nc.vector.tensor_scalar_mul(S2[:], S2[:], 1.0 / CIN)
        nc.vector.tensor_tensor(var[:], S2[:], var[:], mybir.AluOpType.subtract)
        nc.vector.tensor_scalar_add(var[:], var[:], 1e-5)
        nc.scalar.sqrt(var[:], var[:])              # std
        nc.vector.reciprocal_approx_fast(rstd[:], var[:])
        nc.vector.tensor_tensor(sneg[:], mean[:], rstd[:], mybir.AluOpType.mult)
        nc.vector.tensor_scalar_mul(sneg[:], sneg[:], -1.0)
        stat_bf = smallp.tile([CIN, 2 * PCK], bf16, tag="stat_bf")
        nc.vector.tensor_copy(stat_bf[:, 0:PCK], rstd[:])
        nc.vector.tensor_copy(stat_bf[:, PCK:2 * PCK], sneg[:])

        # unpack to rows via DRAM bounce
        r_dram = dramp.tile([2 * PCK * CIN], bf16)
        nc.sync.dma_start(r_dram[0:PCK * CIN].rearrange("(p j) -> p j", p=CIN),
                          stat_bf[:, 0:PCK])
        nc.sync.dma_start(r_dram[PCK * CIN:].rearrange("(p j) -> p j", p=CIN),
                          stat_bf[:, PCK:2 * PCK])
        rstd_row = smallp.tile([1, NPIX], bf16, tag="rstd_row")
        nc.sync.dma_start(rstd_row[:], r_dram[0:NPIX].rearrange("(o p) -> o p", o=1))
        srow2 = smallp.tile([2, NPIX], bf16, tag="srow2")
        nc.sync.dma_start(srow2[0:1, :], r_dram[NPIX:].rearrange("(o p) -> o p", o=1))
        nc.sync.dma_start(srow2[1:2, :], oner_ext[:])

        # ---- padded K/V buffers (zero borders) ----
        k_pad = big.tile([CIN, NPAD], bf16)
        v_pad = big.tile([CIN, NPAD], bf16)
        nc.gpsimd.memset(k_pad[:], 0.0)
        nc.gpsimd.memset(v_pad[:], 0.0)
        q_tiles = [None] * NCHUNK
        xs_tiles = [None] * NCHUNK

        def pad_view(t, c, delta=0):
            # rows c*8 .. c*8+7 of padded buffer, inner 64 cols, shifted by delta
            off = (1 + c * ROWS_PER_CHUNK) * PW + 1 + delta
            return t[:, off:off + ROWS_PER_CHUNK * PW].rearrange(
                "p (r w) -> p r w", r=ROWS_PER_CHUNK, w=PW)[:, :, 0:W]

        AUGQ, AUGK, AUGV, AUGS = (slice(0, CIN), slice(CIN, 2 * CIN),
                                  slice(2 * CIN, 3 * CIN),
                                  slice(3 * CIN, 3 * CIN + NH * K2))

        # ---- phase 2: normalize + projections (fills q_sb, k_pad, v_pad) ----
        for c in range(NCHUNK):
            sl = slice(c * CHUNK, (c + 1) * CHUNK)
            rb = ps128.tile([CIN, CHUNK], f32, tag="ps128")
            mm(rb[:], ones_m[:], rstd_row[:, sl], start=True, stop=True)
            xs_c = chkp.tile([CIN, CHUNK], bf16, tag="xs")
            xs_tiles[c] = xs_c
            nc.vector.tensor_tensor(xs_c[:], x_sb[:, sl], rb[:], mybir.AluOpType.mult)

            qp = ps128.tile([CIN, CHUNK], f32, tag="ps128")
            mm(qp[:], wq[:], xs_c[:], start=True, stop=False)
            mm(qp[:], waug[:, AUGQ], srow2[:, sl], start=False, stop=True)
            q_c = chkp.tile([CIN, CHUNK], bf16, tag="q")
            q_tiles[c] = q_c
            nc.vector.tensor_copy(q_c[:], qp[:])

            kp = ps128.tile([CIN, CHUNK], f32, tag="ps128")
            mm(kp[:], wk[:], xs_c[:], start=True, stop=False)
            mm(kp[:], waug[:, AUGK], srow2[:, sl], start=False, stop=True)
            nc.vector.tensor_copy(pad_view(k_pad, c)[:], kp[:].rearrange(
                "p (r w) -> p r w", r=ROWS_PER_CHUNK, w=W))

            vp = ps128.tile([CIN, CHUNK], f32, tag="ps128")
            mm(vp[:], wv[:], xs_c[:], start=True, stop=False)
            mm(vp[:], waug[:, AUGV], srow2[:, sl], start=False, stop=True)
            nc.scalar.copy(pad_view(v_pad, c)[:], vp[:].rearrange(
                "p (r w) -> p r w", r=ROWS_PER_CHUNK, w=W))

        # ---- phase 3+4 per chunk: scores, softmax, AV, Wf, out ----
        for c in range(NCHUNK):
            sl = slice(c * CHUNK, (c + 1) * CHUNK)
            q_v = q_tiles[c][:].rearrange("p (r w) -> p r w", r=ROWS_PER_CHUNK, w=W)

            sc = ps36.tile([NH * K2, CHUNK], f32, tag="ps36")
            mm(sc[:], wqs[:], xs_tiles[c][:], start=True, stop=False)
            mm(sc[:], waug[:, AUGS], srow2[:, sl], start=False, stop=False)
            for k in range(K2):
                pk = mkp.tile([CIN, CHUNK], bf16, tag="pk")
                pk_v = pk[:].rearrange("p (r w) -> p r w", r=ROWS_PER_CHUNK, w=W)
                eng = nc.gpsimd if k in (0, 5) else nc.vector
                eng.tensor_tensor(pk_v[:], q_v[:],
                                  pad_view(k_pad, c, _shift_delta(k))[:],
                                  mybir.AluOpType.mult)
                mm(sc[:], bdw[:, k * NH * K2:(k + 1) * NH * K2], pk[:],
                   start=False, stop=(k == K2 - 1))

            exp_c = chkp.tile([NH * K2, CHUNK], bf16, tag="exp")
            nc.scalar.activation(exp_c[:], sc[:], AF.Exp)
            dn = ps36.tile([NH * K2, CHUNK], f32, tag="ps36")
            mm(dn[:], rsw[:], exp_c[:], start=True, stop=True)
            rcp36 = mkp.tile([NH * K2, CHUNK], f32, tag="rcp")
            nc.vector.reciprocal_approx_fast(rcp36[:], dn[:])
            attn_c = chkp.tile([NH * K2, CHUNK], bf16, tag="attn")
            nc.vector.tensor_tensor(attn_c[:], exp_c[:], rcp36[:],
                                    mybir.AluOpType.mult)

            acc = psacc.tile([COUT, CHUNK], f32, tag="acc")
            m_all = mallp.tile([CIN, K2 * CHUNK], bf16, tag="mall")
            for k in range(K2):
                rep = ps128.tile([CIN, CHUNK], f32, tag="ps128")
                mm(rep[:], ekw[:, k * CIN:(k + 1) * CIN], attn_c[:],
                   start=True, stop=True)
                rep_sb = mkp.tile([CIN, CHUNK], bf16, tag="repsb")
                nc.scalar.copy(rep_sb[:], rep[:])
                mk_v = m_all[:, k * CHUNK:(k + 1) * CHUNK].rearrange(
                    "p (r w) -> p r w", r=ROWS_PER_CHUNK, w=W)
                nc.vector.tensor_tensor(
                    mk_v[:], rep_sb[:].rearrange("p (r w) -> p r w",
                                                 r=ROWS_PER_CHUNK, w=W),
                    pad_view(v_pad, c, _shift_delta(k))[:],
                    mybir.AluOpType.mult)
                mm(acc[:], wf[:], m_all[:, k * CHUNK:(k + 1) * CHUNK],
                   start=(k == 0), stop=(k == K2 - 1))
            out_sb = mkp.tile([COUT, CHUNK], f32, tag="outsb")
            nc.vector.tensor_scalar_add(out_sb[:], acc[:], bfb[:])
            nc.sync.dma_start(out_ext[:, sl], out_sb[:])


def _get_compiled():
    if "nc" not in _CACHE:
        _CACHE["nc"] = _build_bass()
    return _CACHE["nc"]


def kernel(**inputs):
    x = np.asarray(inputs["x"], dtype=np.float32)          # [B, CIN, H, W]
    consts = _host_fold(
        np.asarray(inputs["ln_g"]), np.asarray(inputs["ln_b"]),
        np.asarray(inputs["Wq"]), np.asarray(inputs["bq"]),
        np.asarray(inputs["Wk"]), np.asarray(inputs["bk"]),
        np.asarray(inputs["Wv"]), np.asarray(inputs["bv"]),
        np.asarray(inputs["Wp"]), np.asarray(inputs["bp"]),
        np.asarray(inputs["Wf"]), np.asarray(inputs["bf"]),
    )

    nc = _get_compiled()

    from concourse.bass_utils import run_bass_kernel_spmd

    core_ids = list(range(B))
    in_maps = []
    for i in range(B):
        m = {"x": np.ascontiguousarray(x[i].reshape(CIN, NPIX))}
        m.update(consts)
        in_maps.append(m)

    res = run_bass_kernel_spmd(nc, in_maps, core_ids,
                               trace=bool(int(os.environ.get("KTRACE", "0"))))
    _CACHE["last_result"] = res
    out = np.stack([res.results[i]["out"].reshape(COUT, H, W) for i in range(B)])
    return out.astype(np.float32)


if __name__ == "__main__":
    # smoke build only
    nc = _get_compiled()
    print("compiled OK")


# revision 20
# speedup vs baseline: 1.1432x; 1.0257x over previous
"""AttentionConv2D (3x3 windowed multi-head attention) on 8 TRN2 NeuronCores.

Sharding: data-parallel over batch (B=8 -> 1 image per core), weights replicated.
Per-core layout: channel-major [128 ch, 4096 pix]. All cross-channel/window
reductions and broadcasts run on the TensorEngine via block-structured matmuls;
per-pixel products run on DVE/ACT.

Math (host-folded):
  xn = (x - mu)/sqrt(var+eps) * g + b        (LayerNorm over channels)
  z  = (x - mu) * rstd;  xn = z*g + b  ->  fold g into W rows, b into biases.
  Q = z @ Wq' + bq'   (Wq' = diag(g) Wq, bq' = b@Wq + bq); same K', V'.
  scores[p,(n,k)] = (Q[p,n,:] . K[p+dk,n,:] + Q[p,n,:] . pos[k,n,:]) * A^-0.5
  pos-term folds to z @ Wqs + bqs (host-computed).
  attn = softmax_k(scores); out = sum_k attn_k * V_shift_k; final = out @ Wf + bf.

Device pipeline per core (z never materialized; aug rank-1 matmuls add the
-mu*rstd correction and biases):
  xs = x * bcast(rstd)          (bf16)
  Qp = Wq'.T @ xs (+aug)        etc.  -> PSUM, evict to SBUF bf16 (K,V padded)
  P_k = Q . K_pad(shift k)      (DVE bf16)
  scores_psum += BD_k.T @ P_k   (PE, accumulates on pos-scores)
  exp_s = exp(scores)           (ACT) -> SBUF
  denom36 = RS36.T @ exp_s      (PE: sum over k, replicated over k rows)
  attn = exp_s * recip(denom36) (DVE)
  rep_k = E_k.T @ attn          (PE: replicate head scores over 32 o-channels)
  m_k = rep_k * V_pad(shift k)  (DVE/ACT)
  final_psum += Wf.T @ m_k      (PE accumulates the sum over k)
  out = final_psum + bf -> DMA out.
"""

import math
import os
import sys

import numpy as np

sys.path.insert(0, "/opt/trn_rl_repo")

import ml_dtypes  # noqa: E402

BF16 = ml_dtypes.bfloat16

B, CIN, COUT, H, W, KS, NH = 8, 128, 128, 64, 64, 3, 4
A = CIN // NH          # 32
OSH = COUT // NH       # 32
K2 = KS * KS           # 9
NPIX = H * W           # 4096
PW = W + 2             # 66 padded width
PH = H + 2
NPAD = PW * PH + PW + 2  # 4356 + slack so shifted strided views stay in-bounds
NCHUNK = 8
CHUNK = NPIX // NCHUNK  # 512
ROWS_PER_CHUNK = H // NCHUNK  # 8
SCALE = A ** (-0.5)

_CACHE = {}


def _pos_encoding_np():
    pos = np.arange(K2, dtype=np.float32)[:, None]
    div = np.exp(np.arange(0, CIN, 2, dtype=np.float32) * (-math.log(10000.0) / CIN))
    ang = pos * div[None, :]
    return np.stack([np.sin(ang), np.cos(ang)], -1).reshape(K2, CIN)


def _host_fold(ln_g, ln_b, Wq, bq, Wk, bk, Wv, bv, Wp, bp, Wf, bf):
    """All weight-space precomputation (f64 for accuracy, cast at the end)."""
    g = ln_g.astype(np.float64)
    b = ln_b.astype(np.float64)
    Wq = Wq.astype(np.float64); Wk = Wk.astype(np.float64)
    Wv = Wv.astype(np.float64); Wp = Wp.astype(np.float64)
    Wf = Wf.astype(np.float64)
    bq = bq.astype(np.float64); bk = bk.astype(np.float64)
    bv = bv.astype(np.float64); bp = bp.astype(np.float64)
    bfv = bf.astype(np.float64)

    Wq_ = g[:, None] * Wq; bq_ = b @ Wq + bq
    Wk_ = g[:, None] * Wk; bk_ = b @ Wk + bk
    Wv_ = g[:, None] * Wv; bv_ = b @ Wv + bv

    pos = _pos_encoding_np().astype(np.float64) @ Wp + bp  # [K2, NH*A]
    pos = pos.reshape(K2, NH, A)

    # pos-scores: row layout (n,k) = n*9+k ; scores_pos = z @ Wqs + bqs, scaled
    Wqs = np.zeros((CIN, NH * K2))
    bqs = np.zeros((NH * K2,))
    Wq_r = Wq_.reshape(CIN, NH, A)
    bq_r = bq_.reshape(NH, A)
    for n in range(NH):
        for k in range(K2):
            Wqs[:, n * K2 + k] = Wq_r[:, n, :] @ pos[k, n, :]
            bqs[n * K2 + k] = bq_r[n, :] @ pos[k, n, :]
    Wqs *= SCALE
    bqs *= SCALE

    # BD_k [CIN, 36]: (n,a) x (n*9+k) = SCALE ; concat over k -> [128, 9*36]
    bd = np.zeros((K2, CIN, NH * K2))
    for k in range(K2):
        for n in range(NH):
            bd[k, n * A:(n + 1) * A, n * K2 + k] = SCALE
    bd = np.concatenate([bd[k] for k in range(K2)], axis=1)  # [128, 324]

    # E_k [36, 128]: (n*9+k') x (n,o) = 1 iff k'==k ; concat -> [36, 9*128]
    ek = np.zeros((K2, NH * K2, CIN))
    for k in range(K2):
        for n in range(NH):
            ek[k, n * K2 + k, n * OSH:(n + 1) * OSH] = 1.0
    ek = np.concatenate([ek[k] for k in range(K2)], axis=1)  # [36, 1152]

    # RS36 [36, 36]: (n*9+k) x (n'*9+k') = 1 iff n==n'  (sum over k, rep over k')
    rs = np.zeros((NH * K2, NH * K2))
    for n in range(NH):
        rs[n * K2:(n + 1) * K2, n * K2:(n + 1) * K2] = 1.0

    # aug lhsT rows: [colsum(W'); bias] per projection, concat cols: q,k,v,qs
    def aug(Wm, bm):
        return np.stack([Wm.sum(axis=0), bm], axis=0)  # [2, M]

    waug = np.concatenate(
        [aug(Wq_, bq_), aug(Wk_, bk_), aug(Wv_, bv_), aug(Wqs, bqs)], axis=1
    )  # [2, 128*3+36]

    c = {
        "wq": Wq_.astype(BF16), "wk": Wk_.astype(BF16), "wv": Wv_.astype(BF16),
        "wqs": Wqs.astype(BF16), "waug": waug.astype(BF16),
        "bd": bd.astype(BF16), "ek": ek.astype(BF16), "rs": rs.astype(BF16),
        "wf": Wf.astype(BF16),
        "bfb": bfv.astype(np.float32).reshape(COUT, 1),
        "ones_k": np.ones((CIN, 1), dtype=BF16),
        "ones_m": np.ones((1, CIN), dtype=BF16),
        "ones_row": np.ones((1, NPIX), dtype=BF16),
    }
    return c


def _shift_delta(k):
    di, dj = k // KS - 1, k % KS - 1
    return di * PW + dj


def _build_bass():
    import concourse.bass as bass
    import concourse.tile as tile
    from concourse import bacc, mybir

    f32 = mybir.dt.float32
    bf16 = mybir.dt.bfloat16
    AF = mybir.ActivationFunctionType

    nc = bacc.Bacc("TRN2", target_bir_lowering=False, debug=False)

    x_ext = nc.dram_tensor("x", [CIN, NPIX], f32, kind="ExternalInput")
    wq_ext = nc.dram_tensor("wq", [CIN, CIN], bf16, kind="ExternalInput")
    wk_ext = nc.dram_tensor("wk", [CIN, CIN], bf16, kind="ExternalInput")
    wv_ext = nc.dram_tensor("wv", [CIN, CIN], bf16, kind="ExternalInput")
    wqs_ext = nc.dram_tensor("wqs", [CIN, NH * K2], bf16, kind="ExternalInput")
    waug_ext = nc.dram_tensor("waug", [2, 3 * CIN + NH * K2], bf16, kind="ExternalInput")
    bd_ext = nc.dram_tensor("bd", [CIN, K2 * NH * K2], bf16, kind="ExternalInput")
    ek_ext = nc.dram_tensor("ek", [NH * K2, K2 * CIN], bf16, kind="ExternalInput")
    rs_ext = nc.dram_tensor("rs", [NH * K2, NH * K2], bf16, kind="ExternalInput")
    wf_ext = nc.dram_tensor("wf", [COUT, COUT], bf16, kind="ExternalInput")
    bfb_ext = nc.dram_tensor("bfb", [COUT, 1], f32, kind="ExternalInput")
    onek_ext = nc.dram_tensor("ones_k", [CIN, 1], bf16, kind="ExternalInput")
    onem_ext = nc.dram_tensor("ones_m", [1, CIN], bf16, kind="ExternalInput")
    oner_ext = nc.dram_tensor("ones_row", [1, NPIX], bf16, kind="ExternalInput")
    out_ext = nc.dram_tensor("out", [COUT, NPIX], f32, kind="ExternalOutput")

    with tile.TileContext(nc) as tc:
        _kernel_body(tc, nc, mybir, f32, bf16, AF, bass,
                     x_ext, wq_ext, wk_ext, wv_ext, wqs_ext, waug_ext, bd_ext,
                     ek_ext, rs_ext, wf_ext, bfb_ext, onek_ext, onem_ext,
                     oner_ext, out_ext)

    nc.compile()
    return nc


def _kernel_body(tc, nc, mybir, f32, bf16, AF, bass,
                 x_ext, wq_ext, wk_ext, wv_ext, wqs_ext, waug_ext, bd_ext,
                 ek_ext, rs_ext, wf_ext, bfb_ext, onek_ext, onem_ext,
                 oner_ext, out_ext):
    from contextlib import ExitStack

    ctx = ExitStack()
    with ctx:
        consts = ctx.enter_context(tc.tile_pool(name="consts", bufs=1))
        big = ctx.enter_context(tc.tile_pool(name="big", bufs=1))
        mkp = ctx.enter_context(tc.tile_pool(name="mk", bufs=6))
        mallp = ctx.enter_context(tc.tile_pool(name="mall", bufs=3))
        chkp = ctx.enter_context(tc.tile_pool(name="chk", bufs=8))
        smallp = ctx.enter_context(tc.tile_pool(name="small", bufs=1))
        dramp = ctx.enter_context(tc.tile_pool(name="drams", bufs=1, space="DRAM"))
        ps128 = ctx.enter_context(tc.tile_pool(name="ps128", bufs=2, space="PSUM"))
        psacc = ctx.enter_context(tc.tile_pool(name="psacc", bufs=2, space="PSUM"))
        ps36 = ctx.enter_context(tc.tile_pool(name="ps36", bufs=4, space="PSUM"))

        def mm(out, lhsT, rhs, **kw):
            nc.tensor.matmul(out, lhsT, rhs, **kw)

        # ---- load constants ----
        wq = consts.tile([CIN, CIN], bf16); nc.sync.dma_start(wq[:], wq_ext[:])
        wk = consts.tile([CIN, CIN], bf16); nc.sync.dma_start(wk[:], wk_ext[:])
        wv = consts.tile([CIN, CIN], bf16); nc.sync.dma_start(wv[:], wv_ext[:])
        wqs = consts.tile([CIN, NH * K2], bf16); nc.sync.dma_start(wqs[:], wqs_ext[:])
        waug = consts.tile([2, 3 * CIN + NH * K2], bf16)
        nc.sync.dma_start(waug[:], waug_ext[:])
        bdw = consts.tile([CIN, K2 * NH * K2], bf16); nc.sync.dma_start(bdw[:], bd_ext[:])
        ekw = consts.tile([NH * K2, K2 * CIN], bf16); nc.sync.dma_start(ekw[:], ek_ext[:])
        rsw = consts.tile([NH * K2, NH * K2], bf16); nc.sync.dma_start(rsw[:], rs_ext[:])
        wf = consts.tile([COUT, COUT], bf16); nc.sync.dma_start(wf[:], wf_ext[:])
        bfb = consts.tile([COUT, 1], f32); nc.sync.dma_start(bfb[:], bfb_ext[:])
        ones_k = consts.tile([CIN, 1], bf16); nc.sync.dma_start(ones_k[:], onek_ext[:])
        ones_m = consts.tile([1, CIN], bf16); nc.sync.dma_start(ones_m[:], onem_ext[:])

        # ---- input image ----
        x_sb = big.tile([CIN, NPIX], f32)
        QT = NPIX // 4
        for _i in range(4):
            nc.sync.dma_start(x_sb[:, _i * QT:(_i + 1) * QT],
                              x_ext[:, _i * QT:(_i + 1) * QT])

        # ---- stats: S1 = sum_c x, S2 = sum_c x^2 (bf16 inputs, fp32 psum) ----
        s1_rows = smallp.tile([1, NPIX], f32, tag="s1_rows")
        s2_rows = smallp.tile([1, NPIX], f32, tag="s2_rows")
        for c in range(NCHUNK):
            sl = slice(c * CHUNK, (c + 1) * CHUNK)
            x_bf = mkp.tile([CIN, CHUNK], bf16, tag="xbf")
            nc.gpsimd.tensor_copy(x_bf[:], x_sb[:, sl])
            sq_bf = mkp.tile([CIN, CHUNK], bf16, tag="sqbf")
            nc.gpsimd.tensor_tensor(sq_bf[:], x_sb[:, sl], x_sb[:, sl],
                                    mybir.AluOpType.mult)
            s1 = ps36.tile([1, CHUNK], f32, tag="ps36")
            mm(s1[:], ones_k[:], x_bf[:], start=True, stop=True)
            s2 = ps36.tile([1, CHUNK], f32, tag="ps36")
            mm(s2[:], ones_k[:], sq_bf[:], start=True, stop=True)
            nc.vector.tensor_copy(s1_rows[0:1, sl], s1[:])
            nc.scalar.copy(s2_rows[0:1, sl], s2[:])

        # ---- pack stats via DRAM bounce -> [128, 64] ----
        s_dram = dramp.tile([2, NPIX], f32)
        nc.sync.dma_start(s_dram[0:1, :], s1_rows[:])
        nc.sync.dma_start(s_dram[1:2, :], s2_rows[:])
        s_pack = smallp.tile([CIN, 2 * NPIX // CIN], f32, tag="s_pack")  # [128, 64]
        PCK = NPIX // CIN  # 32
        nc.sync.dma_start(s_pack[:, 0:PCK], s_dram[0, :].rearrange("(p j) -> p j", p=CIN))
        nc.sync.dma_start(s_pack[:, PCK:2 * PCK], s_dram[1, :].rearrange("(p j) -> p j", p=CIN))

        # rstd = 1/sqrt(S2/128 - (S1/128)^2 + eps); sneg = -(S1/128)*rstd
        S1 = s_pack[:, 0:PCK]
        S2 = s_pack[:, PCK:2 * PCK]
        stat2 = smallp.tile([CIN, 4 * PCK], f32, tag="stat2")
        mean = stat2[:, 0:PCK]
        var = stat2[:, PCK:2 * PCK]
        rstd = stat2[:, 2 * PCK:3 * PCK]
        sneg = stat2[:, 3 * PCK:4 * PCK]
        nc.vector.tensor_scalar_mul(mean[:], S1[:], 1.0 / CIN)
        nc.vector.tensor_tensor(var[:], mean[:], mean[:], mybir.AluOpType.mult)
        nc.vector.tensor_scalar_mul(S2[:], S2[:], 1.0 / CIN)
        nc.vector.tensor_tensor(var[:], S2[:], var[:], mybir.AluOpType.subtract)
        nc.vector.tensor_scalar_add(var[:], var[:], 1e-5)
        nc.scalar.sqrt(var[:], var[:])              # std
        nc.vector.reciprocal_approx_fast(rstd[:], var[:])
        nc.vector.tensor_tensor(sneg[:], mean[:], rstd[:], mybir.AluOpType.mult)
        nc.vector.tensor_scalar_mul(sneg[:], sneg[:], -1.0)
        stat_bf = smallp.tile([CIN, 2 * PCK], bf16, tag="stat_bf")
        nc.vector.tensor_copy(stat_bf[:, 0:PCK], rstd[:])
        nc.vector.tensor_copy(stat_bf[:, PCK:2 * PCK], sneg[:])

        # unpack to rows via DRAM bounce
        r_dram = dramp.tile([2 * PCK * CIN], bf16)
        nc.sync.dma_start(r_dram[0:PCK * CIN].rearrange("(p j) -> p j", p=CIN),
                          stat_bf[:, 0:PCK])
        nc.sync.dma_start(r_dram[PCK * CIN:].rearrange("(p j) -> p j", p=CIN),
                          stat_bf[:, PCK:2 * PCK])
        rstd_row = smallp.tile([1, NPIX], bf16, tag="rstd_row")
        nc.sync.dma_start(rstd_row[:], r_dram[0:NPIX].rearrange("(o p) -> o p", o=1))
        srow2 = smallp.tile([2, NPIX], bf16, tag="srow2")
        nc.sync.dma_start(srow2[0:1, :], r_dram[NPIX:].rearrange("(o p) -> o p", o=1))
        nc.sync.dma_start(srow2[1:2, :], oner_ext[:])

        # ---- padded K/V buffers (zero borders) ----
        k_pad = big.tile([CIN, NPAD], bf16)
        v_pad = big.tile([CIN, NPAD], bf16)
        nc.gpsimd.memset(k_pad[:], 0.0)
        nc.gpsimd.memset(v_pad[:], 0.0)
        q_tiles = [None] * NCHUNK
        xs_tiles = [None] * NCHUNK

        def pad_view(t, c, delta=0):
            # rows c*8 .. c*8+7 of padded buffer, inner 64 cols, shifted by delta
            off = (1 + c * ROWS_PER_CHUNK) * PW + 1 + delta
            return t[:, off:off + ROWS_PER_CHUNK * PW].rearrange(
                "p (r w) -> p r w", r=ROWS_PER_CHUNK, w=PW)[:, :, 0:W]

        AUGQ, AUGK, AUGV, AUGS = (slice(0, CIN), slice(CIN, 2 * CIN),
                                  slice(2 * CIN, 3 * CIN),
                                  slice(3 * CIN, 3 * CIN + NH * K2))

        # ---- phase 2: normalize + projections (fills q_sb, k_pad, v_pad) ----
        for c in range(NCHUNK):
            sl = slice(c * CHUNK, (c + 1) * CHUNK)
            rb = ps128.tile([CIN, CHUNK], f32, tag="ps128")
            mm(rb[:], ones_m[:], rstd_row[:, sl], start=True, stop=True)
            xs_c = chkp.tile([CIN, CHUNK], bf16, tag="xs")
            xs_tiles[c] = xs_c
            nc.vector.tensor_tensor(xs_c[:], x_sb[:, sl], rb[:], mybir.AluOpType.mult)

            qp = ps128.tile([CIN, CHUNK], f32, tag="ps128")
            mm(qp[:], wq[:], xs_c[:], start=True, stop=False)
            mm(qp[:], waug[:, AUGQ], srow2[:, sl], start=False, stop=True)
            q_c = chkp.tile([CIN, CHUNK], bf16, tag="q")
            q_tiles[c] = q_c
            nc.vector.tensor_copy(q_c[:], qp[:])

            kp = ps128.tile([CIN, CHUNK], f32, tag="ps128")
            mm(kp[:], wk[:], xs_c[:], start=True, stop=False)
            mm(kp[:], waug[:, AUGK], srow2[:, sl], start=False, stop=True)
            nc.vector.tensor_copy(pad_view(k_pad, c)[:], kp[:].rearrange(
                "p (r w) -> p r w", r=ROWS_PER_CHUNK, w=W))

            vp = ps128.tile([CIN, CHUNK], f32, tag="ps128")
            mm(vp[:], wv[:], xs_c[:], start=True, stop=False)
            mm(vp[:], waug[:, AUGV], srow2[:, sl], start=False, stop=True)
            nc.scalar.copy(pad_view(v_pad, c)[:], vp[:].rearrange(
                "p (r w) -> p r w", r=ROWS_PER_CHUNK, w=W))

        # ---- phase 3+4 per chunk: scores, softmax, AV, Wf, out ----
        for c in range(NCHUNK):
            sl = slice(c * CHUNK, (c + 1) * CHUNK)
            q_v = q_tiles[c][:].rearrange("p (r w) -> p r w", r=ROWS_PER_CHUNK, w=W)

            sc = ps36.tile([NH * K2, CHUNK], f32, tag="ps36")
            mm(sc[:], wqs[:], xs_tiles[c][:], start=True, stop=False)
            mm(sc[:], waug[:, AUGS], srow2[:, sl], start=False, stop=False)
            for k in range(K2):
                pk = mkp.tile([CIN, CHUNK], bf16, tag="pk")
                pk_v = pk[:].rearrange("p (r w) -> p r w", r=ROWS_PER_CHUNK, w=W)
                eng = nc.gpsimd if k in (0, 5) else nc.vector
                eng.tensor_tensor(pk_v[:], q_v[:],
                                  pad_view(k_pad, c, _shift_delta(k))[:],
                                  mybir.AluOpType.mult)
                mm(sc[:], bdw[:, k * NH * K2:(k + 1) * NH * K2], pk[:],
                   start=False, stop=(k == K2 - 1))

            exp_c = chkp.tile([NH * K2, CHUNK], bf16, tag="exp")
            nc.scalar.activation(exp_c[:], sc[:], AF.Exp)
            dn = ps36.tile([NH * K2, CHUNK], f32, tag="ps36")
            mm(dn[:], rsw[:], exp_c[:], start=True, stop=True)
            rcp36 = mkp.tile([NH * K2, CHUNK], f32, tag="rcp")
            nc.vector.reciprocal_approx_fast(rcp36[:], dn[:])
            attn_c = chkp.tile([NH * K2, CHUNK], bf16, tag="attn")
            nc.vector.tensor_tensor(attn_c[:], exp_c[:], rcp36[:],
                                    mybir.AluOpType.mult)

            acc = psacc.tile([COUT, CHUNK], f32, tag="acc")
            m_all = mallp.tile([CIN, K2 * CHUNK], bf16, tag="mall")
            for k in range(K2):
                rep = ps128.tile([CIN, CHUNK], f32, tag="ps128")
                mm(rep[:], ekw[:, k * CIN:(k + 1) * CIN], attn_c[:],
                   start=True, stop=True)
                rep_sb = mkp.tile([CIN, CHUNK], bf16, tag="repsb")
                nc.scalar.copy(rep_sb[:], rep[:])
                mk_v = m_all[:, k * CHUNK:(k + 1) * CHUNK].rearrange(
                    "p (r w) -> p r w", r=ROWS_PER_CHUNK, w=W)
                nc.vector.tensor_tensor(
                    mk_v[:], rep_sb[:].rearrange("p (r w) -> p r w",
                                                 r=ROWS_PER_CHUNK, w=W),
                    pad_view(v_pad, c, _shift_delta(k))[:],
                    mybir.AluOpType.mult)
                mm(acc[:], wf[:], m_all[:, k * CHUNK:(k + 1) * CHUNK],
                   start=(k == 0), stop=(k == K2 - 1))
            out_sb = mkp.tile([COUT, CHUNK], f32, tag="outsb")
            nc.vector.tensor_scalar_add(out_sb[:], acc[:], bfb[:])
            nc.sync.dma_start(out_ext[:, sl], out_sb[:])


def _get_compiled():
    if "nc" not in _CACHE:
        _CACHE["nc"] = _build_bass()
    return _CACHE["nc"]


def kernel(**inputs):
    x = np.asarray(inputs["x"], dtype=np.float32)          # [B, CIN, H, W]
    consts = _host_fold(
        np.asarray(inputs["ln_g"]), np.asarray(inputs["ln_b"]),
        np.asarray(inputs["Wq"]), np.asarray(inputs["bq"]),
        np.asarray(inputs["Wk"]), np.asarray(inputs["bk"]),
        np.asarray(inputs["Wv"]), np.asarray(inputs["bv"]),
        np.asarray(inputs["Wp"]), np.asarray(inputs["bp"]),
        np.asarray(inputs["Wf"]), np.asarray(inputs["bf"]),
    )

    nc = _get_compiled()

    from concourse.bass_utils import run_bass_kernel_spmd

    core_ids = list(range(B))
    in_maps = []
    for i in range(B):
        m = {"x": np.ascontiguousarray(x[i].reshape(CIN, NPIX))}
        m.update(consts)
        in_maps.append(m)

    res = run_bass_kernel_spmd(nc, in_maps, core_ids,
                               trace=bool(int(os.environ.get("KTRACE", "0"))))
    _CACHE["last_result"] = res
    out = np.stack([res.results[i]["out"].reshape(COUT, H, W) for i in range(B)])
    return out.astype(np.float32)


if __name__ == "__main__":
    # smoke build only
    nc = _get_compiled()
    print("compiled OK")
